# revision 1
# baseline (speedup 1.0000x reference)
"""Trainium2 Bass kernel for nn_GATLayer (2x relational attention, B=8,N=2048,D=256).

Key math: the score Linear(2d->1) on concat decomposes additively, so
score[b,i,j] = qdot[b,i] + kdot[b,j] + bs.  Softmax over j is invariant to
per-row constants, hence attn[b,i,:] = softmax_j(kdot[b,:]) for EVERY i.
The whole attention collapses to per-batch vector work:

  layer(p_in, x_in, mask):                       # kv side = x_in
    e    = exp(x_in @ u) * mask                  # u = Wk @ Ws[d:,0]
    A    = sum(e);  xbar = (e @ x_in) / A
    ctx  = xbar @ Wv + bv                        # (d,) per batch
    g    = sigmoid(p_in @ w + ctx.wg1 + bg)      # w = Wg[:d,0]+Wg[d:,0]
    out  = p_in + g * ctx

  x_new = 2x + g1*ctx1          (layer1: p_in=x, kv=p, no mask)
  p_new = 2p + g2*ctx2          (layer2: p_in=p, kv=x_new, mask)

Layer-2 terms are re-expressed against the ORIGINAL x (never materializing
x_new on the critical path):
  kdot2 = x_new@u2 = 2(x@u2) + (ctx1.u2)*g1
  e2@x_new = 2(e2@x) + (e2.g1)*ctx1

Sharding: data-parallel over batch, one batch per NeuronCore (8 cores).
"""

import numpy as np

B, N, D = 8, 2048, 256
P = 128            # partitions
T = N // P         # 16 tiles of (128, 256) per tensor
NCORES = 8
CHUNK = 4          # DMA / cast granularity in tiles
NS = 10            # tiles 0..NS-1 use the PE path for the output combine,
                   # tiles NS..T-1 use the ACT+DVE path (engine balancing)


def _fold_host(inputs):
    """Fold weights on host (fp64 for accuracy, cast to f32/bf16)."""
    import ml_dtypes

    f = {}
    for L in ("ra1", "ra2"):
        Wk = inputs[f"{L}_Wk"].astype(np.float64)
        Ws = inputs[f"{L}_Ws"].astype(np.float64)
        Wg = inputs[f"{L}_Wg"].astype(np.float64)
        u = Wk @ Ws[D:, 0]                       # (D,)
        w = Wg[:D, 0] + Wg[D:, 0]                # (D,)
        f[f"{L}_u"] = u.astype(np.float32)
        f[f"{L}_w"] = w.astype(np.float32)
        f[f"{L}_wg1"] = Wg[:D, 0].astype(np.float32)
        f[f"{L}_bv"] = inputs[f"{L}_bv"].astype(np.float32)
        f[f"{L}_bg"] = float(inputs[f"{L}_bg"][0])
        f[f"{L}_Wv_bf"] = inputs[f"{L}_Wv"].astype(ml_dtypes.bfloat16)
    return f


def _perm(a):
    # (2048, 256) -> (128, 16*256): partition p holds rows {p, 128+p, ...}
    return np.ascontiguousarray(
        a.reshape(T, P, D).transpose(1, 0, 2).reshape(P, T * D))


def _unperm(a):
    return np.ascontiguousarray(
        a.reshape(P, T, D).transpose(1, 0, 2).reshape(N, D))


def build(inputs):
    """Build the Bass program + per-core input maps.

    Returns (nc, in_maps, post) where post(results) -> (x_new, p_new).
    """
    import ml_dtypes
    import concourse.bacc as bacc
    import concourse.tile as tile
    import concourse.mybir as mybir

    f32 = mybir.dt.float32
    bf16 = mybir.dt.bfloat16
    MUL = mybir.AluOpType.mult
    ADD = mybir.AluOpType.add
    EXP = mybir.ActivationFunctionType.Exp
    SIG = mybir.ActivationFunctionType.Sigmoid
    CPY = mybir.ActivationFunctionType.Copy

    fold = _fold_host(inputs)
    bg1, bg2 = fold["ra1_bg"], fold["ra2_bg"]

    nc = bacc.Bacc()

    # ---- DRAM I/O -------------------------------------------------------
    x_d = nc.dram_tensor("x", [P, T * D], f32, kind="ExternalInput")
    p_d = nc.dram_tensor("p", [P, T * D], f32, kind="ExternalInput")
    m_d = nc.dram_tensor("mask", [P, T], f32, kind="ExternalInput")
    wv1_d = nc.dram_tensor("wv1", [P, 2 * D], bf16, kind="ExternalInput")
    wv2_d = nc.dram_tensor("wv2", [P, 2 * D], bf16, kind="ExternalInput")
    eye_d = nc.dram_tensor("eye", [P, P], f32, kind="ExternalInput")
    twoi_d = nc.dram_tensor("two_i", [P, P], f32, kind="ExternalInput")
    onesrf_d = nc.dram_tensor("ones_r_f", [1, P], f32, kind="ExternalInput")
    onesrb_d = nc.dram_tensor("ones_r_b", [1, P], bf16, kind="ExternalInput")
    onescf_d = nc.dram_tensor("ones_c_f", [P, 1], f32, kind="ExternalInput")
    bgs_d = nc.dram_tensor("bgs", [1, 2], f32, kind="ExternalInput")
    # bf16 rows broadcast on device: u1, 2*u2, w1, w2
    rowsb_d = nc.dram_tensor("rows_b", [1, 4 * D], bf16, kind="ExternalInput")
    # f32 rows used directly: u2, wg11, wg12, bv1, bv2
    rowsf_d = nc.dram_tensor("rows_f", [1, 5 * D], f32, kind="ExternalInput")

    xo_d = nc.dram_tensor("x_out", [P, T * D], f32, kind="ExternalOutput")
    po_d = nc.dram_tensor("p_out", [P, T * D], f32, kind="ExternalOutput")

    with tile.TileContext(nc) as tc:
        with (
            tc.tile_pool(name="big", bufs=1) as big,
            tc.tile_pool(name="small", bufs=1) as small,
            tc.tile_pool(name="ps_g", bufs=3, space="PSUM") as ps_g,
            tc.tile_pool(name="ps_xb", bufs=2, space="PSUM") as ps_xb,
            tc.tile_pool(name="ps_sm", bufs=3, space="PSUM") as ps_sm,
        ):
            # ---- persistent SBUF ----------------------------------------
            x_sb = big.tile([P, T, D], f32)
            p_sb = big.tile([P, T, D], f32)
            x_bf = big.tile([P, T, D], bf16)
            p_bf = big.tile([P, T, D], bf16)
            xn_sb = big.tile([P, T, D], f32)
            pn_sb = big.tile([P, T, D], f32)
            wv1 = big.tile([P, 2, D], bf16)
            wv2 = big.tile([P, 2, D], bf16)
            eye = big.tile([P, P], f32)
            twoi = big.tile([P, P], f32)
            ones_rf = small.tile([1, P], f32)
            ones_rb = small.tile([1, P], bf16)
            ones_cf = small.tile([P, 1], f32)
            rows_b = small.tile([1, 4, D], bf16)
            rows_f = small.tile([1, 5, D], f32)
            mask_sb = small.tile([P, T], f32)
            bgs = small.tile([1, 2], f32)

            # ---- loads --------------------------------------------------
            for ch in range(0, T, CHUNK):
                s = slice(ch * D, (ch + CHUNK) * D)
                nc.sync.dma_start(x_sb[:, ch:ch + CHUNK, :], x_d[:, s])
                nc.sync.dma_start(p_sb[:, ch:ch + CHUNK, :], p_d[:, s])
            nc.sync.dma_start(mask_sb[:], m_d[:])
            nc.sync.dma_start(wv1[:], wv1_d[:])
            nc.sync.dma_start(wv2[:], wv2_d[:])
            nc.sync.dma_start(eye[:], eye_d[:])
            nc.sync.dma_start(twoi[:], twoi_d[:])
            nc.sync.dma_start(ones_rf[:], onesrf_d[:])
            nc.sync.dma_start(ones_rb[:], onesrb_d[:])
            nc.sync.dma_start(ones_cf[:], onescf_d[:])
            nc.sync.dma_start(rows_b[:], rowsb_d[:])
            nc.sync.dma_start(rows_f[:], rowsf_d[:])
            nc.sync.dma_start(bgs[:], bgs_d[:])

            u2_row = rows_f[:, 0, :]
            wg11_row = rows_f[:, 1, :]
            wg12_row = rows_f[:, 2, :]
            bv1_row = rows_f[:, 3, :]
            bv2_row = rows_f[:, 4, :]

            # ---- bf16 casts (DVE for x, ACT for p) ----------------------
            for ch in range(0, T, CHUNK):
                nc.vector.tensor_copy(x_bf[:, ch:ch + CHUNK, :],
                                      x_sb[:, ch:ch + CHUNK, :])
                nc.scalar.copy(p_bf[:, ch:ch + CHUNK, :],
                               p_sb[:, ch:ch + CHUNK, :])

            # ---- broadcast the 4 bf16 weight rows to 128 partitions -----
            wbc = big.tile([P, 4, D], bf16)   # u1b, u2b2, w1b, w2b
            for i in range(4):
                bc_ps = ps_sm.tile([P, D], f32, tag="sm")
                nc.tensor.matmul(bc_ps[:], ones_rb[:], rows_b[:, i, :],
                                 start=True, stop=True)
                nc.scalar.copy(wbc[:, i, :], bc_ps[:])

            # ---- the 4 row-dot passes (DVE, bf16, fused mult+reduce) ----
            sk1 = small.tile([P, T], f32)
            gx1 = small.tile([P, T], f32)
            sx2 = small.tile([P, T], f32)
            gp2 = small.tile([P, T], f32)
            junk = big.tile([P, D], bf16)
            for t in range(T):
                nc.vector.scalar_tensor_tensor(
                    out=junk[:], in0=p_bf[:, t, :], scalar=1.0,
                    in1=wbc[:, 0, :], op0=MUL, op1=MUL,
                    accum_out=sk1[:, t:t + 1])
                nc.vector.scalar_tensor_tensor(
                    out=junk[:], in0=x_bf[:, t, :], scalar=1.0,
                    in1=wbc[:, 1, :], op0=MUL, op1=MUL,
                    accum_out=sx2[:, t:t + 1])
                nc.vector.scalar_tensor_tensor(
                    out=junk[:], in0=x_bf[:, t, :], scalar=1.0,
                    in1=wbc[:, 2, :], op0=MUL, op1=MUL,
                    accum_out=gx1[:, t:t + 1])
                nc.vector.scalar_tensor_tensor(
                    out=junk[:], in0=p_bf[:, t, :], scalar=1.0,
                    in1=wbc[:, 3, :], op0=MUL, op1=MUL,
                    accum_out=gp2[:, t:t + 1])

            # =============== layer 1 attention (kv = p) ==================
            e1f = small.tile([P, T], f32)
            e1b = small.tile([P, T], bf16)
            nc.scalar.activation(e1f[:], sk1[:], EXP)
            nc.vector.tensor_copy(e1b[:], e1f[:])

            # A1 = sum(e1); r1 = 1/A1
            a1_ps = ps_sm.tile([1, T], f32, tag="sm")
            nc.tensor.matmul(a1_ps[:], ones_cf[:], e1f[:], start=True, stop=True)
            a1 = small.tile([1, 1], f32, tag="a1")
            nc.vector.tensor_reduce(a1[:], a1_ps[:], axis=mybir.AxisListType.X,
                                    op=ADD)
            r1 = small.tile([1, 1], f32, tag="r1")
            nc.vector.reciprocal(r1[:], a1[:])

            # xbarT1[d,c] = sum_j e1[j] * p[j, d]  (unnormalized)
            xb1_ps = ps_xb.tile([P, 2], f32, tag="xb")
            for c in range(2):
                for t in range(T):
                    nc.tensor.matmul(
                        xb1_ps[:, c:c + 1],
                        p_bf[:, t, c * P:(c + 1) * P],
                        e1b[:, t:t + 1],
                        start=(t == 0), stop=(t == T - 1))
            xb1 = small.tile([P, 2], bf16, tag="xb1s")
            nc.vector.tensor_copy(xb1[:], xb1_ps[:])

            # ctx1 = xbar1 @ Wv1 / A1 + bv1
            c1_ps = ps_sm.tile([1, D], f32, tag="sm")
            for c in range(2):
                nc.tensor.matmul(c1_ps[:], xb1[:, c:c + 1], wv1[:, c, :],
                                 start=(c == 0), stop=(c == 1))
            ctx1 = small.tile([1, D], f32, tag="ctx1")
            nc.vector.scalar_tensor_tensor(
                out=ctx1[:], in0=c1_ps[:], scalar=r1[:], in1=bv1_row,
                op0=MUL, op1=ADD)
            ctx1_bf = small.tile([1, D], bf16, tag="ctx1b")
            nc.vector.tensor_copy(ctx1_bf[:], ctx1[:])

            # gamma1 = ctx1 . wg11 + bg1 ;  c21 = ctx1 . u2
            jrow = small.tile([1, D], f32, tag="jrow")
            g1g = small.tile([1, 1], f32, tag="g1g")
            nc.vector.scalar_tensor_tensor(
                out=jrow[:], in0=ctx1[:], scalar=1.0, in1=wg11_row,
                op0=MUL, op1=MUL, accum_out=g1g[:])
            c21 = small.tile([1, 1], f32, tag="c21")
            nc.vector.scalar_tensor_tensor(
                out=jrow[:], in0=ctx1[:], scalar=1.0, in1=u2_row,
                op0=MUL, op1=MUL, accum_out=c21[:])

            # broadcast gamma1, c21 across partitions (PE ones trick)
            g1c_ps = ps_sm.tile([P, 1], f32, tag="sm")
            nc.tensor.matmul(g1c_ps[:], ones_rf[:], g1g[:], start=True, stop=False)
            nc.tensor.matmul(g1c_ps[:], ones_rf[:], bgs[:, 0:1], start=False,
                             stop=True)
            g1col = small.tile([P, 1], f32, tag="g1col")
            nc.vector.tensor_copy(g1col[:], g1c_ps[:])
            c21c_ps = ps_sm.tile([P, 1], f32, tag="sm")
            nc.tensor.matmul(c21c_ps[:], ones_rf[:], c21[:], start=True, stop=True)
            c21col = small.tile([P, 1], f32, tag="c21col")
            nc.vector.tensor_copy(c21col[:], c21c_ps[:])

            # g1 = sigmoid(gx1 + gamma1)
            g1 = small.tile([P, T], f32)
            nc.scalar.activation(g1[:], gx1[:], SIG, bias=g1col[:])

            # g1 transposed to rows (for outer products), bf16, flattened to
            # one partition so row slices are PE-legal (base partition 0)
            g1t_ps = ps_sm.tile([T, P], f32, tag="sm")
            nc.tensor.transpose(g1t_ps[:], g1[:], eye[:])
            g1t_sb = small.tile([T, P], bf16, tag="g1ts")
            nc.vector.tensor_copy(g1t_sb[:], g1t_ps[:])
            g1t = small.tile([1, T * P], bf16, tag="g1t")
            nc.gpsimd.dma_start(g1t[:], g1t_sb[:])

            # ctx1 broadcast tile (f32) for the ACT-path output combine
            cb1_ps = ps_sm.tile([P, D], f32, tag="sm")
            nc.tensor.matmul(cb1_ps[:], ones_rb[:], ctx1_bf[:], start=True,
                             stop=True)
            ctx1_bc = big.tile([P, D], f32, tag="ctx1bc")
            nc.scalar.copy(ctx1_bc[:], cb1_ps[:])

            # =============== layer 2 attention (kv = x_new) ==============
            # kdot2 = 2*(x@u2) + c21*g1   (sx2 already includes the 2x fold)
            sk2 = small.tile([P, T], f32)
            nc.vector.scalar_tensor_tensor(
                out=sk2[:], in0=g1[:], scalar=c21col[:], in1=sx2[:],
                op0=MUL, op1=ADD)
            e2f = small.tile([P, T], f32)
            nc.scalar.activation(e2f[:], sk2[:], EXP)
            e2m = small.tile([P, T], f32)
            nc.vector.tensor_tensor(out=e2m[:], in0=e2f[:], in1=mask_sb[:],
                                    op=MUL)
            e2b = small.tile([P, T], bf16)   # 2*e2, bf16
            nc.vector.tensor_scalar(out=e2b[:], in0=e2m[:], scalar1=2.0,
                                    scalar2=None, op0=MUL)

            a2_ps = ps_sm.tile([1, T], f32, tag="sm")
            nc.tensor.matmul(a2_ps[:], ones_cf[:], e2m[:], start=True, stop=True)
            a2 = small.tile([1, 1], f32, tag="a2")
            nc.vector.tensor_reduce(a2[:], a2_ps[:], axis=mybir.AxisListType.X,
                                    op=ADD)
            r2 = small.tile([1, 1], f32, tag="r2")
            nc.vector.reciprocal(r2[:], a2[:])

            # dot22 = sum(e2 * g1) -> cross-partition sum
            jcol = small.tile([P, T], f32, tag="jcol")
            d22p = small.tile([P, 1], f32, tag="d22p")
            nc.vector.scalar_tensor_tensor(
                out=jcol[:], in0=e2m[:], scalar=1.0, in1=g1[:],
                op0=MUL, op1=MUL, accum_out=d22p[:])
            d22_ps = ps_sm.tile([1, 1], f32, tag="sm")
            nc.tensor.matmul(d22_ps[:], ones_cf[:], d22p[:], start=True,
                             stop=True)
            d22 = small.tile([1, 1], bf16, tag="d22")
            nc.vector.tensor_copy(d22[:], d22_ps[:])

            # xbarT2 = (2 e2) @ x + dot22 * ctx1   (unnormalized)
            xb2_ps = ps_xb.tile([P, 2], f32, tag="xb")
            for c in range(2):
                for t in range(T):
                    nc.tensor.matmul(
                        xb2_ps[:, c:c + 1],
                        x_bf[:, t, c * P:(c + 1) * P],
                        e2b[:, t:t + 1],
                        start=(t == 0), stop=False)
                nc.tensor.matmul(
                    xb2_ps[:, c:c + 1],
                    ctx1_bf[:, c * P:(c + 1) * P],
                    d22[:],
                    start=False, stop=True)
            xb2 = small.tile([P, 2], bf16, tag="xb2s")
            nc.vector.tensor_copy(xb2[:], xb2_ps[:])

            c2_ps = ps_sm.tile([1, D], f32, tag="sm")
            for c in range(2):
                nc.tensor.matmul(c2_ps[:], xb2[:, c:c + 1], wv2[:, c, :],
                                 start=(c == 0), stop=(c == 1))
            ctx2 = small.tile([1, D], f32, tag="ctx2")
            nc.vector.scalar_tensor_tensor(
                out=ctx2[:], in0=c2_ps[:], scalar=r2[:], in1=bv2_row,
                op0=MUL, op1=ADD)
            ctx2_bf = small.tile([1, D], bf16, tag="ctx2b")
            nc.vector.tensor_copy(ctx2_bf[:], ctx2[:])

            g2g = small.tile([1, 1], f32, tag="g2g")
            nc.vector.scalar_tensor_tensor(
                out=jrow[:], in0=ctx2[:], scalar=1.0, in1=wg12_row,
                op0=MUL, op1=MUL, accum_out=g2g[:])
            g2c_ps = ps_sm.tile([P, 1], f32, tag="sm")
            nc.tensor.matmul(g2c_ps[:], ones_rf[:], g2g[:], start=True, stop=False)
            nc.tensor.matmul(g2c_ps[:], ones_rf[:], bgs[:, 1:2], start=False,
                             stop=True)
            g2col = small.tile([P, 1], f32, tag="g2col")
            nc.vector.tensor_copy(g2col[:], g2c_ps[:])

            g2 = small.tile([P, T], f32)
            nc.scalar.activation(g2[:], gp2[:], SIG, bias=g2col[:])
            g2t_ps = ps_sm.tile([T, P], f32, tag="sm")
            nc.tensor.transpose(g2t_ps[:], g2[:], eye[:])
            g2t_sb = small.tile([T, P], bf16, tag="g2ts")
            nc.vector.tensor_copy(g2t_sb[:], g2t_ps[:])
            g2t = small.tile([1, T * P], bf16, tag="g2t")
            nc.gpsimd.dma_start(g2t[:], g2t_sb[:])

            cb2_ps = ps_sm.tile([P, D], f32, tag="sm")
            nc.tensor.matmul(cb2_ps[:], ones_rb[:], ctx2_bf[:], start=True,
                             stop=True)
            ctx2_bc = big.tile([P, D], f32, tag="ctx2bc")
            nc.scalar.copy(ctx2_bc[:], cb2_ps[:])

            # =============== output combine + stores =====================
            # x_new = 2x + g1 (x) ctx1 ;  p_new = 2p + g2 (x) ctx2
            for (src, dst, gt, gcols, cbf, cbc, out_d) in (
                (x_sb, xn_sb, g1t, g1, ctx1_bf, ctx1_bc, xo_d),
                (p_sb, pn_sb, g2t, g2, ctx2_bf, ctx2_bc, po_d),
            ):
                for t in range(T):
                    if t < NS:
                        # PE path: psum = 2I @ src + g^T (outer) ctx
                        gp = ps_g.tile([P, D], f32, tag="gps")
                        nc.tensor.matmul(gp[:], twoi[:], src[:, t, :],
                                         start=True, stop=False)
                        nc.tensor.matmul(gp[:], gt[0:1, t * P:(t + 1) * P],
                                         cbf[:], start=False, stop=True)
                        nc.scalar.copy(dst[:, t, :], gp[:])
                    else:
                        # ACT+DVE path: tmp = g*ctx_bc; dst = 2*src + tmp
                        tmp = big.tile([P, D], f32, tag="gtmp")
                        nc.scalar.activation(tmp[:], cbc[:], CPY,
                                             scale=gcols[:, t:t + 1])
                        nc.vector.scalar_tensor_tensor(
                            out=dst[:, t, :], in0=src[:, t, :], scalar=2.0,
                            in1=tmp[:], op0=MUL, op1=ADD)
                for ch in range(0, T, CHUNK):
                    s = slice(ch * D, (ch + CHUNK) * D)
                    nc.sync.dma_start(out_d[:, s], dst[:, ch:ch + CHUNK, :])

    nc.finalize()

    # ---- per-core inputs ------------------------------------------------
    eye_np = np.eye(P, dtype=np.float32)
    shared = {
        "wv1": np.ascontiguousarray(
            fold["ra1_Wv_bf"].reshape(2, P, D).transpose(1, 0, 2).reshape(P, 2 * D)),
        "wv2": np.ascontiguousarray(
            fold["ra2_Wv_bf"].reshape(2, P, D).transpose(1, 0, 2).reshape(P, 2 * D)),
        "eye": eye_np,
        "two_i": 2.0 * eye_np,
        "ones_r_f": np.ones((1, P), np.float32),
        "ones_r_b": np.ones((1, P), ml_dtypes.bfloat16),
        "ones_c_f": np.ones((P, 1), np.float32),
        "bgs": np.array([[fold["ra1_bg"], fold["ra2_bg"]]], np.float32),
        "rows_b": np.concatenate([
            fold["ra1_u"], 2.0 * fold["ra2_u"], fold["ra1_w"], fold["ra2_w"],
        ]).astype(ml_dtypes.bfloat16).reshape(1, 4 * D),
        "rows_f": np.concatenate([
            fold["ra2_u"], fold["ra1_wg1"], fold["ra2_wg1"],
            fold["ra1_bv"], fold["ra2_bv"],
        ]).astype(np.float32).reshape(1, 5 * D),
    }
    x_np = np.asarray(inputs["x"], dtype=np.float32)
    p_np = np.asarray(inputs["p"], dtype=np.float32)
    m_np = np.asarray(inputs["mask"]).astype(np.float32)
    in_maps = []
    for b in range(NCORES):
        im = dict(shared)
        im["x"] = _perm(x_np[b])
        im["p"] = _perm(p_np[b])
        im["mask"] = np.ascontiguousarray(m_np[b].reshape(T, P).T)
        in_maps.append(im)

    def post(results):
        x_new = np.stack([_unperm(results[b]["x_out"]) for b in range(NCORES)])
        p_new = np.stack([_unperm(results[b]["p_out"]) for b in range(NCORES)])
        return x_new, p_new

    return nc, in_maps, post


def kernel(**inputs):
    from concourse.bass_utils import run_bass_kernel_spmd

    nc, in_maps, post = build(inputs)
    res = run_bass_kernel_spmd(nc, in_maps, core_ids=list(range(NCORES)))
    return post(res.results)



# revision 3
# speedup vs baseline: 1.9566x; 1.9566x over previous
"""Trainium2 Bass kernel for nn_GATLayer (2x relational attention, B=8,N=2048,D=256).

Math: the score Linear(2d->1) on concat decomposes additively, so softmax
attention weights are identical for every query row; each attention collapses
to one context vector per batch:

  e1   = exp(p.u1);  A1 = sum(e1);  xbar1 = (e1 @ p)/A1
  ctx1 = xbar1 @ Wv1 + bv1
  g1   = sigmoid(x.w1 + ctx1.wg11 + bg1)        (per row)
  x_new = 2x + g1*ctx1
  e2   = exp(2(x.u2) + (ctx1.u2)*g1) * mask
  xbar2 = (e2 @ x_new)/A2 = (2 e2@x + (e2.g1)*ctx1)/A2
  ctx2 = xbar2 @ Wv2 + bv2;  g2 = sigmoid(p.w2 + ctx2.wg21 + bg2)
  p_new = 2p + g2*ctx2

Implementation strategy (one batch per NeuronCore, 8 cores):
 - host sends X2=2x, P2=2p in bf16, both row-form (j on partitions) and
   transposed (d on partitions).  Row dots run on the TENSOR engine using the
   transposed copies: lhsT = X2T[d-half, j-tile] (128x128), rhs = weight pairs
   (128x2) -> per-row dots accumulate in PSUM over the two d-halves.
 - row-form tiles carry extra columns: P2 gets a ones column (col 256) so the
   weighted-sum matmul also produces A1; X2 gets ones (256) and g1 (257) so
   one matmul chain yields sum(e2*x), A2, and sum(e2*g1).
 - ctx matmul rhs carries extra columns Wv@wg1 and Wv@u2 so gamma and c21 fall
   out of the same accumulation; only reciprocal+scale remain as scalar work.
 - combine x_new = ctx1_bc * g1 + X2 is a single DVE scalar_tensor_tensor in
   bf16 (2x mode); a few p_new tiles go via PE outer-product to balance.
 - outputs stored bf16 (tolerance 2e-2 >> bf16 rounding), halving store DMA.
"""

import numpy as np

B, N, D = 8, 2048, 256
P = 128            # partitions
T = N // P         # 16 tiles
H = 2              # d-halves
CH = 4             # tiles per DMA/compute chunk
NCH = T // CH
NCORES = 8
NEG = -1.0e9
PCMB_PE = 6        # p_new tiles T-PCMB_PE..T-1 combined via PE outer product


def _fold_host(inputs):
    f = {}
    for L in ("ra1", "ra2"):
        Wk = inputs[f"{L}_Wk"].astype(np.float64)
        Ws = inputs[f"{L}_Ws"].astype(np.float64)
        Wg = inputs[f"{L}_Wg"].astype(np.float64)
        Wv = inputs[f"{L}_Wv"].astype(np.float64)
        bv = inputs[f"{L}_bv"].astype(np.float64)
        bg = float(inputs[f"{L}_bg"][0])
        f[f"{L}_u"] = Wk @ Ws[D:, 0]
        f[f"{L}_w"] = Wg[:D, 0] + Wg[D:, 0]
        f[f"{L}_wg1"] = Wg[:D, 0]
        f[f"{L}_Wv"] = Wv
        f[f"{L}_bv"] = bv
        f[f"{L}_bg"] = bg
    return f


def _perm_rows(a):
    # (2048, C) -> (128, 16, C): row n = 128*t + j -> [j, t, :]
    C = a.shape[1]
    return a.reshape(T, P, C).transpose(1, 0, 2)


def _perm_T(a):
    # (2048, 256) -> (128, 2, 16, 128): [d', h, t, j] = a[128t+j, 128h+d']
    return a.reshape(T, P, H, P).transpose(3, 2, 0, 1)


def build(inputs):
    import ml_dtypes
    import concourse.bacc as bacc
    import concourse.tile as tile
    import concourse.mybir as mybir

    bf16 = ml_dtypes.bfloat16
    f32 = mybir.dt.float32
    bfd = mybir.dt.bfloat16
    MUL = mybir.AluOpType.mult
    ADD = mybir.AluOpType.add
    EXP = mybir.ActivationFunctionType.Exp
    SIG = mybir.ActivationFunctionType.Sigmoid

    fold = _fold_host(inputs)

    nc = bacc.Bacc()

    # ---- DRAM I/O -------------------------------------------------------
    p2_d = nc.dram_tensor("p2", [P, T * 257], bfd, kind="ExternalInput")
    x2_d = nc.dram_tensor("x2", [P, T * 258], bfd, kind="ExternalInput")
    p2T_d = nc.dram_tensor("p2T", [P, H * T * P], bfd, kind="ExternalInput")
    x2T_d = nc.dram_tensor("x2T", [P, H * T * P], bfd, kind="ExternalInput")
    mlog_d = nc.dram_tensor("mlog", [P, T], f32, kind="ExternalInput")
    wstk_d = nc.dram_tensor("wstk", [P, H * 4], bfd, kind="ExternalInput")
    rhs1_d = nc.dram_tensor("rhs1", [P, H * 258], bfd, kind="ExternalInput")
    rhs2_d = nc.dram_tensor("rhs2", [P, H * 257], bfd, kind="ExternalInput")
    eye_d = nc.dram_tensor("eye", [P, P], bfd, kind="ExternalInput")
    onesb_d = nc.dram_tensor("onesb", [1, P], bfd, kind="ExternalInput")
    onesf_d = nc.dram_tensor("onesf", [1, P], f32, kind="ExternalInput")
    bv1_d = nc.dram_tensor("bv1r", [1, D], f32, kind="ExternalInput")
    bv2_d = nc.dram_tensor("bv2r", [1, D], f32, kind="ExternalInput")
    c1_d = nc.dram_tensor("consts1", [1, 2], f32, kind="ExternalInput")
    c2_d = nc.dram_tensor("consts2", [1, 1], f32, kind="ExternalInput")

    xo_d = nc.dram_tensor("x_out", [P, T * D], bfd, kind="ExternalOutput")
    po_d = nc.dram_tensor("p_out", [P, T * D], bfd, kind="ExternalOutput")

    with tile.TileContext(nc) as tc:
        with (
            tc.tile_pool(name="big", bufs=1) as big,
            tc.tile_pool(name="small", bufs=1) as small,
            tc.tile_pool(name="ps_dot", bufs=2, space="PSUM") as ps_dot,
            tc.tile_pool(name="ps_w", bufs=1, space="PSUM") as ps_w,
            tc.tile_pool(name="ps_sm", bufs=2, space="PSUM") as ps_sm,
            tc.tile_pool(name="ps_bc", bufs=1, space="PSUM") as ps_bc,
            tc.tile_pool(name="ps_cmb", bufs=2, space="PSUM") as ps_cmb,
        ):
            # ---- persistent SBUF ---------------------------------------
            p2 = big.tile([P, T, 257], bfd)
            x2 = big.tile([P, T, 258], bfd)
            p2T = big.tile([P, H, T * P], bfd)
            x2T = big.tile([P, H, T * P], bfd)
            xn = big.tile([P, T, D], bfd)
            pn = big.tile([P, T, D], bfd)
            wstk = small.tile([P, H, 4], bfd)
            rhs1 = small.tile([P, H, 258], bfd)
            rhs2 = small.tile([P, H, 257], bfd)
            eye = small.tile([P, P], bfd)
            onesb = small.tile([1, P], bfd)
            onesf = small.tile([1, P], f32)
            mlog = small.tile([P, T], f32)
            bv1r = small.tile([1, D], f32)
            bv2r = small.tile([1, D], f32)
            consts1 = small.tile([1, 2], f32)
            consts2 = small.tile([1, 1], f32)

            dpp = small.tile([P, T, 2], f32)   # p dots: (u1h, w2h)
            dpx = small.tile([P, T, 2], f32)   # x dots: (u2, w1h)
            e1b = small.tile([P, T], bfd)
            e2b = small.tile([P, T], bfd)
            g1 = small.tile([P, T], f32)
            g2 = small.tile([P, T], f32)
            g1c = small.tile([P, T], bfd)
            g2c = small.tile([P, T], bfd)
            sk2p = small.tile([P, T], f32)
            sk2 = small.tile([P, T], f32)
            gcol = small.tile([P, 4], f32)     # 0=gam1, 1=c21, 2=gam2
            r1 = small.tile([1, 1], f32)
            r2 = small.tile([1, 1], f32)
            d22 = small.tile([1, 1], f32)
            gc1 = small.tile([1, 2], f32)
            gc2 = small.tile([1, 1], f32)
            xb1rb = small.tile([1, D], bfd)
            xb2r = small.tile([1, D], f32)
            xb2rb = small.tile([1, D], bfd)
            xb1T = small.tile([P, 2], bfd)
            xb2T = small.tile([P, 2], bfd)
            ctx1r = small.tile([1, D], f32)
            ctx2r = small.tile([1, D], f32)
            ctx1rb = small.tile([1, D], bfd)
            ctx2rb = small.tile([1, D], bfd)
            ctx1bc = small.tile([P, D], bfd)
            ctx2bc = small.tile([P, D], bfd)
            g2ts = small.tile([T, P], bfd)
            g2t = small.tile([1, T * P], bfd)

            # ---- small loads -------------------------------------------
            nc.sync.dma_start(wstk[:], wstk_d[:])
            nc.sync.dma_start(rhs1[:], rhs1_d[:])
            nc.sync.dma_start(rhs2[:], rhs2_d[:])
            nc.sync.dma_start(eye[:], eye_d[:])
            nc.sync.dma_start(onesb[:], onesb_d[:])
            nc.sync.dma_start(onesf[:], onesf_d[:])
            nc.sync.dma_start(mlog[:], mlog_d[:])
            nc.sync.dma_start(bv1r[:], bv1_d[:])
            nc.sync.dma_start(bv2r[:], bv2_d[:])
            nc.sync.dma_start(consts1[:], c1_d[:])
            nc.sync.dma_start(consts2[:], c2_d[:])

            # ---- big loads (p first, then x; T-form then row-form) -----
            for c in range(NCH):
                for h in range(H):
                    s = slice(h * T * P + c * CH * P, h * T * P + (c + 1) * CH * P)
                    nc.sync.dma_start(p2T[:, h, c * CH * P:(c + 1) * CH * P],
                                      p2T_d[:, s])
                nc.sync.dma_start(p2[:, c * CH:(c + 1) * CH, :],
                                  p2_d[:, c * CH * 257:(c + 1) * CH * 257])
            for c in range(NCH):
                for h in range(H):
                    s = slice(h * T * P + c * CH * P, h * T * P + (c + 1) * CH * P)
                    nc.sync.dma_start(x2T[:, h, c * CH * P:(c + 1) * CH * P],
                                      x2T_d[:, s])
                nc.sync.dma_start(x2[:, c * CH:(c + 1) * CH, :],
                                  x2_d[:, c * CH * 258:(c + 1) * CH * 258])

            w1_ps = ps_w.tile([1, 257], f32, tag="w")

            # ---- helpers ------------------------------------------------
            def dots(c, src_T, dst, wsl):
                """Per-row dots for chunk c: dst[:, t, :] = src.T tile @ wstk."""
                dp_ps = ps_dot.tile([P, CH, 2], f32, tag="dot")
                for i in range(CH):
                    t = c * CH + i
                    for h in range(H):
                        nc.tensor.matmul(
                            dp_ps[:, i, :],
                            src_T[:, h, t * P:(t + 1) * P],
                            wstk[:, h, wsl],
                            start=(h == 0), stop=(h == 1))
                nc.scalar.copy(dst[:, c * CH:(c + 1) * CH, :], dp_ps[:])

            def wsum(c, eb, rows, out_ps, n):
                for i in range(CH):
                    t = c * CH + i
                    nc.tensor.matmul(
                        out_ps[:], eb[:, t:t + 1], rows[:, t, :n],
                        start=(t == 0), stop=(t == T - 1))

            # ---- p-phase: dots + e1 + weighted sums --------------------
            # (emission order guides the static PE program: keep one chunk
            #  of dot-MMs between a chunk's e1 and its wsum MMs)
            def p_chunk_pre(c):
                dots(c, p2T, dpp, slice(0, 2))
                nc.scalar.activation(e1b[:, c * CH:(c + 1) * CH],
                                     dpp[:, c * CH:(c + 1) * CH, 0], EXP)

            def x_chunk_pre(c):
                dots(c, x2T, dpx, slice(2, 4))
                # sk2_pre = 2x.u2 + masklog
                nc.vector.tensor_tensor(
                    out=sk2p[:, c * CH:(c + 1) * CH],
                    in0=dpx[:, c * CH:(c + 1) * CH, 0],
                    in1=mlog[:, c * CH:(c + 1) * CH], op=ADD)

            p_chunk_pre(0)
            p_chunk_pre(1)
            wsum(0, e1b, p2, w1_ps, 257)
            p_chunk_pre(2)
            wsum(1, e1b, p2, w1_ps, 257)
            p_chunk_pre(3)
            wsum(2, e1b, p2, w1_ps, 257)
            wsum(3, e1b, p2, w1_ps, 257)

            # ---- ctx1 chain --------------------------------------------
            nc.vector.reciprocal(r1[:], w1_ps[0:1, 256:257])
            nc.vector.tensor_copy(xb1rb[:], w1_ps[0:1, 0:256])
            xbT_ps = ps_sm.tile([P, 2], f32, tag="sm")
            for h in range(H):
                nc.tensor.matmul(xbT_ps[:, h:h + 1],
                                 xb1rb[0:1, h * P:(h + 1) * P],
                                 onesb[0:1, 0:1], start=True, stop=True)
            nc.vector.tensor_copy(xb1T[:], xbT_ps[:])
            c1_ps = ps_sm.tile([1, 258], f32, tag="sm")
            for h in range(H):
                nc.tensor.matmul(c1_ps[:], xb1T[:, h:h + 1], rhs1[:, h, :],
                                 start=(h == 0), stop=(h == 1))
            nc.vector.scalar_tensor_tensor(
                out=ctx1r[:], in0=c1_ps[0:1, 0:256], scalar=r1[:],
                in1=bv1r[:], op0=MUL, op1=ADD)
            nc.vector.scalar_tensor_tensor(
                out=gc1[:], in0=c1_ps[0:1, 256:258], scalar=r1[:],
                in1=consts1[:], op0=MUL, op1=ADD)
            gr_ps = ps_sm.tile([P, 2], f32, tag="sm")
            nc.tensor.matmul(gr_ps[:], onesf[:], gc1[:], start=True, stop=True)
            nc.vector.tensor_copy(gcol[:, 0:2], gr_ps[:])
            nc.vector.tensor_copy(ctx1rb[:], ctx1r[:])
            bc_ps = ps_bc.tile([P, D], f32, tag="bc")
            nc.tensor.matmul(bc_ps[:], onesb[:], ctx1rb[:], start=True, stop=True)
            nc.scalar.copy(ctx1bc[:], bc_ps[:])

            # ---- x-phase ------------------------------------------------
            x_chunk_pre(0)
            x_chunk_pre(1)

            def x_chunk_post(c):
                cs = slice(c * CH, (c + 1) * CH)
                # g1 = sigmoid(x.w1 + gam1)
                nc.scalar.activation(g1[:, cs], dpx[:, cs, 1], SIG,
                                     bias=gcol[:, 0:1])
                nc.vector.tensor_copy(g1c[:, cs], g1[:, cs])
                nc.vector.tensor_copy(x2[:, cs, 257], g1c[:, cs])
                # x_new tiles + store
                for i in range(CH):
                    t = c * CH + i
                    nc.vector.scalar_tensor_tensor(
                        out=xn[:, t, :], in0=ctx1bc[:], scalar=g1[:, t:t + 1],
                        in1=x2[:, t, 0:256], op0=MUL, op1=ADD)
                nc.scalar.dma_start(xo_d[:, c * CH * D:(c + 1) * CH * D],
                                    xn[:, cs, :])
                # e2 = exp(2x.u2 + mlog + c21*g1)
                nc.vector.scalar_tensor_tensor(
                    out=sk2[:, cs], in0=g1[:, cs], scalar=gcol[:, 1:2],
                    in1=sk2p[:, cs], op0=MUL, op1=ADD)
                nc.scalar.activation(e2b[:, cs], sk2[:, cs], EXP)

            x_chunk_post(0)
            w2_ps = ps_w.tile([1, 258], f32, tag="w")
            x_chunk_pre(2)
            x_chunk_post(1)
            wsum(0, e2b, x2, w2_ps, 258)
            x_chunk_pre(3)
            x_chunk_post(2)
            wsum(1, e2b, x2, w2_ps, 258)
            x_chunk_post(3)
            wsum(2, e2b, x2, w2_ps, 258)
            wsum(3, e2b, x2, w2_ps, 258)

            # ---- ctx2 chain --------------------------------------------
            nc.vector.reciprocal(r2[:], w2_ps[0:1, 256:257])
            nc.vector.tensor_copy(d22[:], w2_ps[0:1, 257:258])
            nc.vector.scalar_tensor_tensor(
                out=xb2r[:], in0=ctx1r[:], scalar=d22[:],
                in1=w2_ps[0:1, 0:256], op0=MUL, op1=ADD)
            nc.vector.tensor_copy(xb2rb[:], xb2r[:])
            xbT2_ps = ps_sm.tile([P, 2], f32, tag="sm")
            for h in range(H):
                nc.tensor.matmul(xbT2_ps[:, h:h + 1],
                                 xb2rb[0:1, h * P:(h + 1) * P],
                                 onesb[0:1, 0:1], start=True, stop=True)
            nc.vector.tensor_copy(xb2T[:], xbT2_ps[:])
            c2_ps = ps_sm.tile([1, 257], f32, tag="sm")
            for h in range(H):
                nc.tensor.matmul(c2_ps[:], xb2T[:, h:h + 1], rhs2[:, h, :],
                                 start=(h == 0), stop=(h == 1))
            nc.vector.scalar_tensor_tensor(
                out=ctx2r[:], in0=c2_ps[0:1, 0:256], scalar=r2[:],
                in1=bv2r[:], op0=MUL, op1=ADD)
            nc.vector.scalar_tensor_tensor(
                out=gc2[:], in0=c2_ps[0:1, 256:257], scalar=r2[:],
                in1=consts2[:], op0=MUL, op1=ADD)
            gr2_ps = ps_sm.tile([P, 1], f32, tag="sm")
            nc.tensor.matmul(gr2_ps[:], onesf[:], gc2[:], start=True, stop=True)
            nc.vector.tensor_copy(gcol[:, 2:3], gr2_ps[:])
            nc.vector.tensor_copy(ctx2rb[:], ctx2r[:])
            bc2_ps = ps_bc.tile([P, D], f32, tag="bc")
            nc.tensor.matmul(bc2_ps[:], onesb[:], ctx2rb[:], start=True, stop=True)
            nc.scalar.copy(ctx2bc[:], bc2_ps[:])

            # ---- g2 + p_new combine ------------------------------------
            nc.scalar.activation(g2[:], dpp[:, :, 1], SIG, bias=gcol[:, 2:3])
            nc.vector.tensor_copy(g2c[:], g2[:])
            g2t_ps = ps_sm.tile([T, P], bfd, tag="sm")
            nc.tensor.transpose(g2t_ps[:], g2c[:], eye[:])
            nc.vector.tensor_copy(g2ts[:], g2t_ps[:])
            nc.gpsimd.dma_start(g2t[:], g2ts[:])

            for t in range(T):
                if t >= T - PCMB_PE:
                    gp = ps_cmb.tile([P, D], f32, tag="cmb")
                    nc.tensor.matmul(gp[:], g2t[0:1, t * P:(t + 1) * P],
                                     ctx2rb[:], start=True, stop=False)
                    nc.tensor.matmul(gp[:], eye[:], p2[:, t, 0:256],
                                     start=False, stop=True)
                    nc.scalar.copy(pn[:, t, :], gp[:])
                else:
                    nc.vector.scalar_tensor_tensor(
                        out=pn[:, t, :], in0=ctx2bc[:], scalar=g2[:, t:t + 1],
                        in1=p2[:, t, 0:256], op0=MUL, op1=ADD)
            for c in range(NCH):
                nc.scalar.dma_start(po_d[:, c * CH * D:(c + 1) * CH * D],
                                    pn[:, c * CH:(c + 1) * CH, :])

    nc.finalize()

    # ---- host-side input prep ------------------------------------------
    u1, w1 = fold["ra1_u"], fold["ra1_w"]
    u2, w2 = fold["ra2_u"], fold["ra2_w"]
    Wv1, Wv2 = fold["ra1_Wv"], fold["ra2_Wv"]
    bv1, bv2 = fold["ra1_bv"], fold["ra2_bv"]
    wg11, wg21 = fold["ra1_wg1"], fold["ra2_wg1"]
    bg1, bg2 = fold["ra1_bg"], fold["ra2_bg"]

    Wv1h = Wv1 / 2.0
    R1 = np.concatenate([Wv1h, (Wv1h @ wg11)[:, None], (Wv1h @ u2)[:, None]],
                        axis=1)                       # (256, 258)
    R2 = np.concatenate([Wv2, (Wv2 @ wg21)[:, None]], axis=1)  # (256, 257)
    ws = np.stack([u1 / 2.0, w2 / 2.0, u2, w1 / 2.0], axis=1)  # (256, 4)

    import ml_dtypes
    bf = ml_dtypes.bfloat16
    shared = {
        "wstk": np.ascontiguousarray(
            ws.reshape(H, P, 4).transpose(1, 0, 2).reshape(P, H * 4)).astype(bf),
        "rhs1": np.ascontiguousarray(
            R1.reshape(H, P, 258).transpose(1, 0, 2).reshape(P, H * 258)).astype(bf),
        "rhs2": np.ascontiguousarray(
            R2.reshape(H, P, 257).transpose(1, 0, 2).reshape(P, H * 257)).astype(bf),
        "eye": np.eye(P).astype(bf),
        "onesb": np.ones((1, P), bf),
        "onesf": np.ones((1, P), np.float32),
        "bv1r": bv1.astype(np.float32).reshape(1, D),
        "bv2r": bv2.astype(np.float32).reshape(1, D),
        "consts1": np.array([[bv1 @ wg11 + bg1, bv1 @ u2]], np.float32),
        "consts2": np.array([[bv2 @ wg21 + bg2]], np.float32),
    }

    x_np = np.asarray(inputs["x"], dtype=np.float64)
    p_np = np.asarray(inputs["p"], dtype=np.float64)
    m_np = np.asarray(inputs["mask"])
    in_maps = []
    ones_col = np.ones((P, T, 1), np.float64)
    for b in range(NCORES):
        X2 = 2.0 * x_np[b]
        P2 = 2.0 * p_np[b]
        x2r = _perm_rows(X2)                      # (128, 16, 256)
        p2r = _perm_rows(P2)
        x2full = np.concatenate([x2r, ones_col, np.zeros((P, T, 1))], axis=2)
        p2full = np.concatenate([p2r, ones_col], axis=2)
        ml = np.where(m_np[b] == 0, NEG, 0.0).astype(np.float32)
        im = dict(shared)
        im["x2"] = np.ascontiguousarray(x2full.reshape(P, T * 258)).astype(bf)
        im["p2"] = np.ascontiguousarray(p2full.reshape(P, T * 257)).astype(bf)
        im["x2T"] = np.ascontiguousarray(_perm_T(X2).reshape(P, H * T * P)).astype(bf)
        im["p2T"] = np.ascontiguousarray(_perm_T(P2).reshape(P, H * T * P)).astype(bf)
        im["mlog"] = np.ascontiguousarray(ml.reshape(T, P).T)
        in_maps.append(im)

    def post(results):
        def unperm(a):
            return np.ascontiguousarray(
                a.astype(np.float32).reshape(P, T, D).transpose(1, 0, 2)
                .reshape(N, D))
        x_new = np.stack([unperm(results[b]["x_out"]) for b in range(NCORES)])
        p_new = np.stack([unperm(results[b]["p_out"]) for b in range(NCORES)])
        return x_new, p_new

    return nc, in_maps, post


def kernel(**inputs):
    from concourse.bass_utils import run_bass_kernel_spmd

    nc, in_maps, post = build(inputs)
    res = run_bass_kernel_spmd(nc, in_maps, core_ids=list(range(NCORES)))
    return post(res.results)


# revision 6
# speedup vs baseline: 2.3052x; 1.1781x over previous
"""Trainium2 Bass kernel for nn_GATLayer (2x relational attention, B=8,N=2048,D=256).

Math: the score Linear(2d->1) on concat decomposes additively, so softmax
attention weights are identical for every query row; each attention collapses
to one context vector per batch:

  e1   = exp(p.u1);  A1 = sum(e1);  xbar1 = (e1 @ p)/A1
  ctx1 = xbar1 @ Wv1 + bv1;  g1 = sigmoid(x.w1 + ctx1.wg11 + bg1)
  x_new = 2x + g1*ctx1
  e2   = exp(2(x.u2) + (ctx1.u2)*g1) * mask
  xbar2 = (2 e2@x + (e2.g1)*ctx1)/A2
  ctx2 = xbar2 @ Wv2 + bv2;  g2 = sigmoid(p.w2 + ctx2.wg21 + bg2)
  p_new = 2p + g2*ctx2

Implementation (one batch per NeuronCore, 8 cores):
 - host sends X2=2x, P2=2p in bf16, row-form (j on partitions) AND transposed
   (d on partitions).  Per-row dots run on the tensor engine off the
   transposed copies (lhsT = X2T tile, rhs = weight pairs).
 - row-form tiles carry extra columns (ones, g1) so the weighted-sum matmuls
   also emit A1, A2 and sum(e2*g1); the ctx matmul rhs carries Wv@wg1 and
   Wv@u2 columns so gamma/c21 fall out of the same accumulation.
 - sigmoids are computed as 1/(1+exp(-z)) so the ONLY activation table set
   used is exp (avoids ~1.3-2.6us table reloads per exp<->sigmoid switch).
 - combine x_new = ctx1_bc * g1 + X2 is one DVE scalar_tensor_tensor in bf16
   (2x mode); some p_new tiles go via PE outer product to balance engines.
 - small constants ride in two consolidated blobs (DMA issue costs ~0.6us
   per descriptor batch on the sync queue); outputs stored bf16.
"""

import numpy as np

B, N, D = 8, 2048, 256
P = 128
T = N // P         # 16 tiles
H = 2              # d-halves
CH = 4             # tiles per compute chunk
NCH = T // CH
NCORES = 8
NEG = -1.0e9
PCMB_PE = 6        # p_new tiles combined via PE outer product

# bf16 blob layout (columns)
WSTK0 = 0          # [P, 8]  : (h, k) -> col h*4+k ; k: u1/2, w2/2, u2, w1/2
RHS1_0 = 8         # [P, 516]: (h, n) -> 8 + h*258 + n
RHS2_0 = 524       # [P, 514]: (h, n) -> 524 + h*257 + n
EYE0 = 1038        # [P, 128]
BLOBB_W = 1166
# f32 blob layout  [1, 515]
BV1_0, BV2_0, CG1N, CC21, CG2N, BLOBF_W = 0, 256, 512, 513, 514, 515


def _fold_host(inputs):
    f = {}
    for L in ("ra1", "ra2"):
        Wk = inputs[f"{L}_Wk"].astype(np.float64)
        Ws = inputs[f"{L}_Ws"].astype(np.float64)
        Wg = inputs[f"{L}_Wg"].astype(np.float64)
        f[f"{L}_u"] = Wk @ Ws[D:, 0]
        f[f"{L}_w"] = Wg[:D, 0] + Wg[D:, 0]
        f[f"{L}_wg1"] = Wg[:D, 0]
        f[f"{L}_Wv"] = inputs[f"{L}_Wv"].astype(np.float64)
        f[f"{L}_bv"] = inputs[f"{L}_bv"].astype(np.float64)
        f[f"{L}_bg"] = float(inputs[f"{L}_bg"][0])
    return f


def _perm_rows(a):
    C = a.shape[1]
    return a.reshape(T, P, C).transpose(1, 0, 2)


def _perm_T(a):
    # (2048, 256) -> (128, 2, 16, 128): [d', h, t, j] = a[128t+j, 128h+d']
    return a.reshape(T, P, H, P).transpose(3, 2, 0, 1)


def build(inputs):
    import ml_dtypes
    import concourse.bacc as bacc
    import concourse.tile as tile
    import concourse.mybir as mybir

    f32 = mybir.dt.float32
    bfd = mybir.dt.bfloat16
    MUL = mybir.AluOpType.mult
    ADD = mybir.AluOpType.add
    EXP = mybir.ActivationFunctionType.Exp

    fold = _fold_host(inputs)
    nc = bacc.Bacc()

    # ---- DRAM I/O -------------------------------------------------------
    p2_d = nc.dram_tensor("p2", [P, T * 257], bfd, kind="ExternalInput")
    x2_d = nc.dram_tensor("x2", [P, T * 258], bfd, kind="ExternalInput")
    p2T_d = nc.dram_tensor("p2T", [P, H * T * P], bfd, kind="ExternalInput")
    x2T_d = nc.dram_tensor("x2T", [P, H * T * P], bfd, kind="ExternalInput")
    mlog_d = nc.dram_tensor("mlog", [P, T], f32, kind="ExternalInput")
    blobb_d = nc.dram_tensor("blobb", [P, BLOBB_W], bfd, kind="ExternalInput")
    blobf_d = nc.dram_tensor("blobf", [1, BLOBF_W], f32, kind="ExternalInput")

    xo_d = nc.dram_tensor("x_out", [P, T * D], bfd, kind="ExternalOutput")
    po_d = nc.dram_tensor("p_out", [P, T * D], bfd, kind="ExternalOutput")

    with tile.TileContext(nc) as tc:
        with (
            tc.tile_pool(name="big", bufs=1) as big,
            tc.tile_pool(name="small", bufs=1) as small,
            tc.tile_pool(name="ps_dot", bufs=2, space="PSUM") as ps_dot,
            tc.tile_pool(name="ps_w", bufs=1, space="PSUM") as ps_w,
            tc.tile_pool(name="ps_sm", bufs=2, space="PSUM") as ps_sm,
            tc.tile_pool(name="ps_bc", bufs=1, space="PSUM") as ps_bc,
            tc.tile_pool(name="ps_cmb", bufs=2, space="PSUM") as ps_cmb,
        ):
            # ---- SBUF ---------------------------------------------------
            p2 = big.tile([P, T, 257], bfd)
            x2 = big.tile([P, T, 258], bfd)
            p2T = big.tile([P, H, T * P], bfd)
            x2T = big.tile([P, H, T * P], bfd)
            xn = big.tile([P, T, D], bfd)
            pn = big.tile([P, T, D], bfd)
            blobb = small.tile([P, BLOBB_W], bfd)
            blobf = small.tile([1, BLOBF_W], f32)
            mlog = small.tile([P, T], f32)
            onesb = small.tile([1, P], bfd)
            onesf = small.tile([1, P], f32)

            dpp = small.tile([P, T, 2], f32)
            dpx = small.tile([P, T, 2], f32)
            e1b = small.tile([P, T], bfd)
            e2b = small.tile([P, T], bfd)
            g1 = small.tile([P, T], f32)
            g2 = small.tile([P, T], f32)
            g2b = small.tile([P, T], bfd)  # g2 in bf16 for transpose
            g2c = small.tile([T, P], bfd)  # evac of g2 transpose
            t1 = small.tile([P, T], f32)
            t2 = small.tile([P, T], f32)
            sk2p = small.tile([P, T], f32)
            sk2 = small.tile([P, T], f32)
            gcol = small.tile([P, 4], f32)   # 0=-gam1, 1=c21, 2=-gam2
            r1 = small.tile([1, 1], f32)
            r1n = small.tile([1, 1], f32)
            r2 = small.tile([1, 1], f32)
            r2n = small.tile([1, 1], f32)
            d22 = small.tile([1, 1], f32)
            gc1 = small.tile([1, 2], f32)
            gc2 = small.tile([1, 1], f32)
            xb1rb = small.tile([1, D], bfd)
            xb2r = small.tile([1, D], f32)
            xb1T = small.tile([P, 2], bfd)
            xb2T = small.tile([P, 2], bfd)
            ctx1r = small.tile([1, D], f32)
            ctx2r = small.tile([1, D], f32)
            ctx1rb = small.tile([1, D], bfd)
            ctx2rb = small.tile([1, D], bfd)
            ctx1bc = small.tile([P, D], bfd)
            ctx2bc = small.tile([P, D], bfd)
            g2t = small.tile([1, T * P], bfd)

            def wslice(h, a, b):
                return blobb[:, WSTK0 + h * 4 + a:WSTK0 + h * 4 + b]

            def rhs1s(h):
                return blobb[:, RHS1_0 + h * 258:RHS1_0 + (h + 1) * 258]

            def rhs2s(h):
                return blobb[:, RHS2_0 + h * 257:RHS2_0 + (h + 1) * 257]

            eye = blobb[:, EYE0:EYE0 + P]
            bv1r = blobf[:, BV1_0:BV1_0 + D]
            bv2r = blobf[:, BV2_0:BV2_0 + D]
            cg1n = blobf[:, CG1N:CG1N + 1]
            cc21 = blobf[:, CC21:CC21 + 1]
            cg2n = blobf[:, CG2N:CG2N + 1]

            # ---- loads --------------------------------------------------
            nc.sync.dma_start(blobb[:], blobb_d[:])
            nc.sync.dma_start(blobf[:], blobf_d[:])
            nc.sync.dma_start(mlog[:], mlog_d[:])
            nc.gpsimd.memset(onesb[:], 1.0)
            nc.gpsimd.memset(onesf[:], 1.0)
            HT = T * P // 2  # half-tensor j-extent per d-half
            for c in range(2):
                for h in range(H):
                    s = slice(h * T * P + c * HT, h * T * P + (c + 1) * HT)
                    nc.sync.dma_start(p2T[:, h, c * HT:(c + 1) * HT], p2T_d[:, s])
                nc.sync.dma_start(p2[:, c * 8:(c + 1) * 8, :],
                                  p2_d[:, c * 8 * 257:(c + 1) * 8 * 257])
            for c in range(2):
                for h in range(H):
                    s = slice(h * T * P + c * HT, h * T * P + (c + 1) * HT)
                    nc.sync.dma_start(x2T[:, h, c * HT:(c + 1) * HT], x2T_d[:, s])
                nc.sync.dma_start(x2[:, c * 8:(c + 1) * 8, :],
                                  x2_d[:, c * 8 * 258:(c + 1) * 8 * 258])

            w1_ps = ps_w.tile([1, 257], f32, tag="w")

            def dots(c, srcT, dst, a, b):
                dp_ps = ps_dot.tile([P, CH, 2], f32, tag="dot")
                for i in range(CH):
                    t = c * CH + i
                    for h in range(H):
                        nc.tensor.matmul(dp_ps[:, i, :],
                                         srcT[:, h, t * P:(t + 1) * P],
                                         wslice(h, a, b),
                                         start=(h == 0), stop=(h == 1))
                nc.scalar.copy(dst[:, c * CH:(c + 1) * CH, :], dp_ps[:])

            def wsum(c, eb, rows, out_ps, n):
                for i in range(CH):
                    t = c * CH + i
                    nc.tensor.matmul(out_ps[:], eb[:, t:t + 1], rows[:, t, :n],
                                     start=(t == 0), stop=(t == T - 1))

            # ---- p-phase ------------------------------------------------
            def p_pre(c):
                cs = slice(c * CH, (c + 1) * CH)
                dots(c, p2T, dpp, 0, 2)
                nc.scalar.activation(e1b[:, cs], dpp[:, cs, 0], EXP)

            def x_pre(c):
                cs = slice(c * CH, (c + 1) * CH)
                dots(c, x2T, dpx, 2, 4)
                nc.vector.tensor_tensor(out=sk2p[:, cs], in0=dpx[:, cs, 0],
                                        in1=mlog[:, cs], op=ADD)

            p_pre(0)
            p_pre(1)
            wsum(0, e1b, p2, w1_ps, 257)
            p_pre(2)
            wsum(1, e1b, p2, w1_ps, 257)
            p_pre(3)
            wsum(2, e1b, p2, w1_ps, 257)
            wsum(3, e1b, p2, w1_ps, 257)

            # ---- ctx1 chain ---------------------------------------------
            nc.vector.reciprocal(r1[:], w1_ps[0:1, 256:257])
            nc.vector.tensor_scalar_mul(out=r1n[:], in0=r1[:], scalar1=-1.0)
            nc.vector.tensor_copy(xb1rb[:], w1_ps[0:1, 0:256])
            xbT_ps = ps_sm.tile([P, 2], f32, tag="sm")
            for h in range(H):
                nc.tensor.matmul(xbT_ps[:, h:h + 1],
                                 xb1rb[0:1, h * P:(h + 1) * P],
                                 onesb[0:1, 0:1], start=True, stop=True)
            nc.vector.tensor_copy(xb1T[:], xbT_ps[:])
            c1_ps = ps_sm.tile([1, 258], f32, tag="sm")
            for h in range(H):
                nc.tensor.matmul(c1_ps[:], xb1T[:, h:h + 1], rhs1s(h),
                                 start=(h == 0), stop=(h == 1))
            nc.vector.scalar_tensor_tensor(
                out=ctx1r[:], in0=c1_ps[0:1, 0:256], scalar=r1[:],
                in1=bv1r, op0=MUL, op1=ADD)
            # gc1 = [-gam1, +c21]
            nc.vector.scalar_tensor_tensor(
                out=gc1[0:1, 0:1], in0=c1_ps[0:1, 256:257], scalar=r1n[:],
                in1=cg1n, op0=MUL, op1=ADD)
            nc.vector.scalar_tensor_tensor(
                out=gc1[0:1, 1:2], in0=c1_ps[0:1, 257:258], scalar=r1[:],
                in1=cc21, op0=MUL, op1=ADD)
            gr_ps = ps_sm.tile([P, 2], f32, tag="sm")
            nc.tensor.matmul(gr_ps[:], onesf[:], gc1[:], start=True, stop=True)
            nc.vector.tensor_copy(gcol[:, 0:2], gr_ps[:])
            nc.vector.tensor_copy(ctx1rb[:], ctx1r[:])
            bc_ps = ps_bc.tile([P, D], f32, tag="bc")
            nc.tensor.matmul(bc_ps[:], onesb[:], ctx1rb[:], start=True, stop=True)
            nc.scalar.copy(ctx1bc[:], bc_ps[:])

            # ---- x-phase ------------------------------------------------
            x_pre(0)
            x_pre(1)

            def x_post(c):
                cs = slice(c * CH, (c + 1) * CH)
                # g1 = 1/(1+exp(-(x.w1) - gam1))
                nc.scalar.activation(t1[:, cs], dpx[:, cs, 1], EXP,
                                     bias=gcol[:, 0:1], scale=-1.0)
                nc.vector.tensor_scalar_add(out=t1[:, cs], in0=t1[:, cs],
                                            scalar1=1.0)
                nc.vector.reciprocal(g1[:, cs], t1[:, cs])
                nc.vector.tensor_copy(x2[:, cs, 257], g1[:, cs])
                nc.vector.scalar_tensor_tensor(
                    out=sk2[:, cs], in0=g1[:, cs], scalar=gcol[:, 1:2],
                    in1=sk2p[:, cs], op0=MUL, op1=ADD)
                nc.scalar.activation(e2b[:, cs], sk2[:, cs], EXP)
                for i in range(CH):
                    t = c * CH + i
                    nc.vector.scalar_tensor_tensor(
                        out=xn[:, t, :], in0=ctx1bc[:], scalar=g1[:, t:t + 1],
                        in1=x2[:, t, 0:256], op0=MUL, op1=ADD)
                nc.sync.dma_start(xo_d[:, c * CH * D:(c + 1) * CH * D],
                                  xn[:, cs, :])

            x_post(0)
            w2_ps = ps_w.tile([1, 258], f32, tag="w")
            x_pre(2)
            x_post(1)
            wsum(0, e2b, x2, w2_ps, 258)
            x_pre(3)
            x_post(2)
            wsum(1, e2b, x2, w2_ps, 258)
            x_post(3)
            wsum(2, e2b, x2, w2_ps, 258)
            wsum(3, e2b, x2, w2_ps, 258)

            # ---- ctx2 chain ---------------------------------------------
            nc.vector.reciprocal(r2[:], w2_ps[0:1, 256:257])
            nc.vector.tensor_scalar_mul(out=r2n[:], in0=r2[:], scalar1=-1.0)
            nc.vector.tensor_copy(d22[:], w2_ps[0:1, 257:258])
            nc.vector.scalar_tensor_tensor(
                out=xb2r[:], in0=ctx1r[:], scalar=d22[:],
                in1=w2_ps[0:1, 0:256], op0=MUL, op1=ADD)
            xbT2_ps = ps_sm.tile([P, 2], f32, tag="sm")
            for h in range(H):
                nc.tensor.matmul(xbT2_ps[:, h:h + 1],
                                 xb2r[0:1, h * P:(h + 1) * P],
                                 onesf[0:1, 0:1], start=True, stop=True)
            nc.vector.tensor_copy(xb2T[:], xbT2_ps[:])
            c2_ps = ps_sm.tile([1, 257], f32, tag="sm")
            for h in range(H):
                nc.tensor.matmul(c2_ps[:], xb2T[:, h:h + 1], rhs2s(h),
                                 start=(h == 0), stop=(h == 1))
            nc.vector.scalar_tensor_tensor(
                out=ctx2r[:], in0=c2_ps[0:1, 0:256], scalar=r2[:],
                in1=bv2r, op0=MUL, op1=ADD)
            nc.vector.scalar_tensor_tensor(
                out=gc2[:], in0=c2_ps[0:1, 256:257], scalar=r2n[:],
                in1=cg2n, op0=MUL, op1=ADD)
            gr2_ps = ps_sm.tile([P, 1], f32, tag="sm")
            nc.tensor.matmul(gr2_ps[:], onesf[:], gc2[:], start=True, stop=True)
            nc.vector.tensor_copy(gcol[:, 2:3], gr2_ps[:])
            nc.vector.tensor_copy(ctx2rb[:], ctx2r[:])
            bc2_ps = ps_bc.tile([P, D], f32, tag="bc")
            nc.tensor.matmul(bc2_ps[:], onesb[:], ctx2rb[:], start=True, stop=True)
            nc.scalar.copy(ctx2bc[:], bc2_ps[:])

            # ---- g2 + p_new ---------------------------------------------
            nc.scalar.activation(t2[:], dpp[:, :, 1], EXP,
                                 bias=gcol[:, 2:3], scale=-1.0)
            nc.vector.tensor_scalar_add(out=t2[:], in0=t2[:], scalar1=1.0)
            nc.vector.reciprocal(g2[:], t2[:])
            nc.vector.tensor_copy(g2b[:], g2[:])
            g2t_ps = ps_sm.tile([T, P], bfd, tag="sm")
            nc.tensor.transpose(g2t_ps[:], g2b[:], eye)
            nc.vector.tensor_copy(g2c[:], g2t_ps[:])
            nc.gpsimd.dma_start(g2t[:], g2c[:])

            for t in range(T):
                if t >= T - PCMB_PE:
                    gp = ps_cmb.tile([P, D], f32, tag="cmb")
                    nc.tensor.matmul(gp[:], g2t[0:1, t * P:(t + 1) * P],
                                     ctx2rb[:], start=True, stop=False)
                    nc.tensor.matmul(gp[:], eye, p2[:, t, 0:256],
                                     start=False, stop=True)
                    nc.scalar.copy(pn[:, t, :], gp[:])
                else:
                    nc.vector.scalar_tensor_tensor(
                        out=pn[:, t, :], in0=ctx2bc[:], scalar=g2[:, t:t + 1],
                        in1=p2[:, t, 0:256], op0=MUL, op1=ADD)
            nc.sync.dma_start(po_d[:, 0:10 * D], pn[:, 0:10, :])
            nc.sync.dma_start(po_d[:, 10 * D:T * D], pn[:, 10:T, :])

    nc.finalize()

    # ---- host-side input prep ------------------------------------------
    u1, w1 = fold["ra1_u"], fold["ra1_w"]
    u2, w2 = fold["ra2_u"], fold["ra2_w"]
    Wv1, Wv2 = fold["ra1_Wv"], fold["ra2_Wv"]
    bv1, bv2 = fold["ra1_bv"], fold["ra2_bv"]
    wg11, wg21 = fold["ra1_wg1"], fold["ra2_wg1"]
    bg1, bg2 = fold["ra1_bg"], fold["ra2_bg"]

    Wv1h = Wv1 / 2.0
    R1 = np.concatenate([Wv1h, (Wv1h @ wg11)[:, None], (Wv1h @ u2)[:, None]],
                        axis=1)
    R2 = np.concatenate([Wv2, (Wv2 @ wg21)[:, None]], axis=1)
    ws = np.stack([u1 / 2.0, w2 / 2.0, u2, w1 / 2.0], axis=1)  # (256, 4)

    import ml_dtypes
    bf = ml_dtypes.bfloat16

    blobb = np.zeros((P, BLOBB_W), np.float64)
    blobb[:, WSTK0:WSTK0 + 8] = ws.reshape(H, P, 4).transpose(1, 0, 2).reshape(P, 8)
    blobb[:, RHS1_0:RHS1_0 + 516] = (
        R1.reshape(H, P, 258).transpose(1, 0, 2).reshape(P, 516))
    blobb[:, RHS2_0:RHS2_0 + 514] = (
        R2.reshape(H, P, 257).transpose(1, 0, 2).reshape(P, 514))
    blobb[:, EYE0:EYE0 + P] = np.eye(P)

    blobf = np.zeros((1, BLOBF_W), np.float32)
    blobf[0, BV1_0:BV1_0 + D] = bv1
    blobf[0, BV2_0:BV2_0 + D] = bv2
    blobf[0, CG1N] = -(bv1 @ wg11 + bg1)
    blobf[0, CC21] = bv1 @ u2
    blobf[0, CG2N] = -(bv2 @ wg21 + bg2)

    shared = {"blobb": blobb.astype(bf), "blobf": blobf}

    x_np = np.asarray(inputs["x"], dtype=np.float64)
    p_np = np.asarray(inputs["p"], dtype=np.float64)
    m_np = np.asarray(inputs["mask"])
    ones_col = np.ones((P, T, 1), np.float64)
    zero_col = np.zeros((P, T, 1), np.float64)
    in_maps = []
    for b in range(NCORES):
        X2 = 2.0 * x_np[b]
        P2 = 2.0 * p_np[b]
        x2full = np.concatenate([_perm_rows(X2), ones_col, zero_col], axis=2)
        p2full = np.concatenate([_perm_rows(P2), ones_col], axis=2)
        ml = np.where(m_np[b] == 0, NEG, 0.0).astype(np.float32)
        im = dict(shared)
        im["x2"] = np.ascontiguousarray(x2full.reshape(P, T * 258)).astype(bf)
        im["p2"] = np.ascontiguousarray(p2full.reshape(P, T * 257)).astype(bf)
        im["x2T"] = np.ascontiguousarray(_perm_T(X2).reshape(P, H * T * P)).astype(bf)
        im["p2T"] = np.ascontiguousarray(_perm_T(P2).reshape(P, H * T * P)).astype(bf)
        im["mlog"] = np.ascontiguousarray(ml.reshape(T, P).T)
        in_maps.append(im)

    def post(results):
        def unperm(a):
            return np.ascontiguousarray(
                a.astype(np.float32).reshape(P, T, D).transpose(1, 0, 2)
                .reshape(N, D))
        x_new = np.stack([unperm(results[b]["x_out"]) for b in range(NCORES)])
        p_new = np.stack([unperm(results[b]["p_out"]) for b in range(NCORES)])
        return x_new, p_new

    return nc, in_maps, post


def kernel(**inputs):
    from concourse.bass_utils import run_bass_kernel_spmd

    nc, in_maps, post = build(inputs)
    res = run_bass_kernel_spmd(nc, in_maps, core_ids=list(range(NCORES)))
    return post(res.results)


# revision 7
# speedup vs baseline: 2.4342x; 1.0560x over previous
"""Trainium2 Bass kernel for nn_GATLayer (2x relational attention, B=8,N=2048,D=256).

Math: the score Linear(2d->1) on concat decomposes additively, so softmax
attention weights are identical for every query row; each attention collapses
to one context vector per batch:

  e1   = exp(p.u1);  A1 = sum(e1);  xbar1 = (e1 @ p)/A1
  ctx1 = xbar1 @ Wv1 + bv1;  g1 = sigmoid(x.w1 + ctx1.wg11 + bg1)
  x_new = 2x + g1*ctx1
  e2   = exp(2(x.u2) + (ctx1.u2)*g1) * mask
  xbar2 = (2 e2@x + (e2.g1)*ctx1)/A2
  ctx2 = xbar2 @ Wv2 + bv2;  g2 = sigmoid(p.w2 + ctx2.wg21 + bg2)
  p_new = 2p + g2*ctx2

Implementation (one batch per NeuronCore, 8 cores):
 - host sends X2=2x, P2=2p in bf16, row-form (j on partitions) AND transposed
   (d on partitions).  Per-row dots run on the tensor engine off the
   transposed copies (lhsT = X2T tile, rhs = weight pairs).
 - row-form tiles carry extra columns (ones, g1) so the weighted-sum matmuls
   also emit A1, A2 and sum(e2*g1); the ctx matmul rhs carries Wv@wg1 and
   Wv@u2 columns so gamma/c21 fall out of the same accumulation.
 - sigmoids are 1/(1+exp(-z)): single activation table set (exp) all kernel.
 - combine x_new = ctx1_bc * g1 + X2 is one DVE scalar_tensor_tensor in bf16
   (2x mode); first p_new tiles go via PE outer product to balance engines.
 - every chunked tensor is a SEPARATE pool tile: the Tile scheduler tracks
   dependencies at tile granularity, so per-chunk tiles let compute start as
   soon as its own chunk's DMA lands (one shared tile serializes on the last
   chunk's DMA).
 - outputs stored bf16 (tolerance 2e-2 >> bf16 rounding), halving store DMA.
"""

import numpy as np

B, N, D = 8, 2048, 256
P = 128
T = N // P         # 16 tiles
H = 2              # d-halves
CH = 4             # tiles per compute chunk
NCH = T // CH
Q = 8              # tiles per DMA chunk (row-form); 2 DMAs per tensor
NCORES = 8
NEG = -1.0e9
PCMB_PE = 5        # p_new tiles 0..PCMB_PE-1 via PE outer product

# bf16 blob layout (columns)
WSTK0 = 0          # [P, 8]  : (h, k) -> col h*4+k ; k: u1/2, w2/2, u2, w1/2
RHS1_0 = 8         # [P, 516]: (h, n) -> 8 + h*258 + n
RHS2_0 = 524       # [P, 514]: (h, n) -> 524 + h*257 + n
EYE0 = 1038        # [P, 128]
BLOBB_W = 1166
# f32 blob layout [P, 531]: mlog in cols 0..15 (all rows); row-0 tables after
MLOG0, BV1_0, BV2_0, CG1N, CC21, CG2N, BLOBF_W = 0, 16, 272, 528, 529, 530, 531


def _fold_host(inputs):
    f = {}
    for L in ("ra1", "ra2"):
        Wk = inputs[f"{L}_Wk"].astype(np.float64)
        Ws = inputs[f"{L}_Ws"].astype(np.float64)
        Wg = inputs[f"{L}_Wg"].astype(np.float64)
        f[f"{L}_u"] = Wk @ Ws[D:, 0]
        f[f"{L}_w"] = Wg[:D, 0] + Wg[D:, 0]
        f[f"{L}_wg1"] = Wg[:D, 0]
        f[f"{L}_Wv"] = inputs[f"{L}_Wv"].astype(np.float64)
        f[f"{L}_bv"] = inputs[f"{L}_bv"].astype(np.float64)
        f[f"{L}_bg"] = float(inputs[f"{L}_bg"][0])
    return f


def _perm_rows(a):
    C = a.shape[1]
    return a.reshape(T, P, C).transpose(1, 0, 2)


def _perm_T(a):
    # (2048, 256) -> (128, 2, 16, 128): [d', h, t, j] = a[128t+j, 128h+d']
    return a.reshape(T, P, H, P).transpose(3, 2, 0, 1)


def build(inputs):
    import ml_dtypes
    import concourse.bacc as bacc
    import concourse.tile as tile
    import concourse.mybir as mybir

    f32 = mybir.dt.float32
    bfd = mybir.dt.bfloat16
    MUL = mybir.AluOpType.mult
    ADD = mybir.AluOpType.add
    EXP = mybir.ActivationFunctionType.Exp

    fold = _fold_host(inputs)
    nc = bacc.Bacc()

    # ---- DRAM I/O -------------------------------------------------------
    p2_d = nc.dram_tensor("p2", [P, T * 257], bfd, kind="ExternalInput")
    x2_d = nc.dram_tensor("x2", [P, T * 258], bfd, kind="ExternalInput")
    p2T_d = nc.dram_tensor("p2T", [P, H * T * P], bfd, kind="ExternalInput")
    x2T_d = nc.dram_tensor("x2T", [P, H * T * P], bfd, kind="ExternalInput")
    blobb_d = nc.dram_tensor("blobb", [P, BLOBB_W], bfd, kind="ExternalInput")
    blobf_d = nc.dram_tensor("blobf", [P, BLOBF_W], f32, kind="ExternalInput")

    xo_d = nc.dram_tensor("x_out", [P, T * D], bfd, kind="ExternalOutput")
    po_d = nc.dram_tensor("p_out", [P, T * D], bfd, kind="ExternalOutput")

    with tile.TileContext(nc) as tc:
        with (
            tc.tile_pool(name="big", bufs=1) as big,
            tc.tile_pool(name="small", bufs=1) as small,
            tc.tile_pool(name="ps_dot", bufs=2, space="PSUM") as ps_dot,
            tc.tile_pool(name="ps_w", bufs=1, space="PSUM") as ps_w,
            tc.tile_pool(name="ps_sm", bufs=2, space="PSUM") as ps_sm,
            tc.tile_pool(name="ps_bc", bufs=1, space="PSUM") as ps_bc,
            tc.tile_pool(name="ps_cmb", bufs=2, space="PSUM") as ps_cmb,
        ):
            # ---- SBUF (per-chunk tiles for dependency granularity) ------
            p2q = [big.tile([P, Q, 257], bfd, name=f"p2q{q}") for q in range(2)]
            x2q = [big.tile([P, Q, 258], bfd, name=f"x2q{q}") for q in range(2)]
            # transposed halves: [h][q] tile of [P, Q*P]
            p2T = [[big.tile([P, Q * P], bfd, name=f"p2T{h}{q}")
                    for q in range(2)] for h in range(H)]
            x2T = [[big.tile([P, Q * P], bfd, name=f"x2T{h}{q}")
                    for q in range(2)] for h in range(H)]
            xnc = [big.tile([P, CH, D], bfd, name=f"xn{c}") for c in range(NCH)]
            pnq = [big.tile([P, Q, D], bfd, name=f"pn{q}") for q in range(2)]
            blobb = small.tile([P, BLOBB_W], bfd)
            blobf = small.tile([P, BLOBF_W], f32)
            onesb = small.tile([1, P], bfd)
            onesf = small.tile([1, P], f32)

            dpp = small.tile([P, T, 2], f32)
            dpxc = [small.tile([P, CH, 2], f32, name=f"dpx{c}")
                    for c in range(NCH)]
            e1c = [small.tile([P, CH], bfd, name=f"e1c{c}") for c in range(NCH)]
            e2c = [small.tile([P, CH], bfd, name=f"e2c{c}") for c in range(NCH)]
            g1c = [small.tile([P, CH], f32, name=f"g1c{c}") for c in range(NCH)]
            t1c = [small.tile([P, CH], f32, name=f"t1c{c}") for c in range(NCH)]
            skpc = [small.tile([P, CH], f32, name=f"skp{c}") for c in range(NCH)]
            skc = [small.tile([P, CH], f32, name=f"skc{c}") for c in range(NCH)]
            g2 = small.tile([P, T], f32)
            g2b = small.tile([P, T], bfd)
            g2c = small.tile([T, P], bfd)
            t2 = small.tile([P, T], f32)
            gcol = small.tile([P, 4], f32)   # 0=-gam1, 1=c21, 2=-gam2
            r1 = small.tile([1, 1], f32)
            r1n = small.tile([1, 1], f32)
            r2 = small.tile([1, 1], f32)
            r2n = small.tile([1, 1], f32)
            d22 = small.tile([1, 1], f32)
            gc1 = small.tile([1, 2], f32)
            gc2 = small.tile([1, 1], f32)
            xb1rb = small.tile([1, D], bfd)
            xb2r = small.tile([1, D], f32)
            xb1T = small.tile([P, 2], bfd)
            xb2T = small.tile([P, 2], bfd)
            ctx1r = small.tile([1, D], f32)
            ctx2r = small.tile([1, D], f32)
            ctx1rb = small.tile([1, D], bfd)
            ctx2rb = small.tile([1, D], bfd)
            ctx1bc = small.tile([P, D], bfd)
            ctx2bc = small.tile([P, D], bfd)
            g2t = small.tile([1, T * P], bfd)

            def wslice(h, a, b):
                return blobb[:, WSTK0 + h * 4 + a:WSTK0 + h * 4 + b]

            def rhs1s(h):
                return blobb[:, RHS1_0 + h * 258:RHS1_0 + (h + 1) * 258]

            def rhs2s(h):
                return blobb[:, RHS2_0 + h * 257:RHS2_0 + (h + 1) * 257]

            eye = blobb[:, EYE0:EYE0 + P]
            mlog = blobf[:, MLOG0:MLOG0 + T]
            bv1r = blobf[0:1, BV1_0:BV1_0 + D]
            bv2r = blobf[0:1, BV2_0:BV2_0 + D]
            cg1n = blobf[0:1, CG1N:CG1N + 1]
            cc21 = blobf[0:1, CC21:CC21 + 1]
            cg2n = blobf[0:1, CG2N:CG2N + 1]

            # ---- loads (p2T halves first so dots start ASAP) ------------
            nc.gpsimd.memset(onesb[:], 1.0)
            nc.gpsimd.memset(onesf[:], 1.0)
            nc.sync.dma_start(blobb[:], blobb_d[:])

            def loadT(dst, dram, h, q):
                s = slice(h * T * P + q * Q * P, h * T * P + (q + 1) * Q * P)
                nc.sync.dma_start(dst[h][q][:], dram[:, s])

            loadT(p2T, p2T_d, 0, 0)
            loadT(p2T, p2T_d, 1, 0)
            nc.sync.dma_start(p2q[0][:], p2_d[:, 0:Q * 257])
            nc.sync.dma_start(blobf[:], blobf_d[:])
            loadT(p2T, p2T_d, 0, 1)
            loadT(p2T, p2T_d, 1, 1)
            nc.sync.dma_start(p2q[1][:], p2_d[:, Q * 257:T * 257])
            loadT(x2T, x2T_d, 0, 0)
            loadT(x2T, x2T_d, 1, 0)
            nc.sync.dma_start(x2q[0][:], x2_d[:, 0:Q * 258])
            loadT(x2T, x2T_d, 0, 1)
            loadT(x2T, x2T_d, 1, 1)
            nc.sync.dma_start(x2q[1][:], x2_d[:, Q * 258:T * 258])

            w1_ps = ps_w.tile([1, 257], f32, tag="w")

            def dots(c, srcT, dst_ap, a, b):
                q, r = divmod(c, 2)
                dp_ps = ps_dot.tile([P, CH, 2], f32, tag="dot")
                for i in range(CH):
                    j0 = (r * CH + i) * P
                    for h in range(H):
                        nc.tensor.matmul(dp_ps[:, i, :],
                                         srcT[h][q][:, j0:j0 + P],
                                         wslice(h, a, b),
                                         start=(h == 0), stop=(h == 1))
                nc.scalar.copy(dst_ap, dp_ps[:])

            def wsum(c, eb, rows_q, out_ps, n, w):
                q, r = divmod(c, 2)
                for i in range(CH):
                    t = c * CH + i
                    nc.tensor.matmul(out_ps[:], eb[:, i:i + 1],
                                     rows_q[q][:, r * CH + i, :n],
                                     start=(t == 0), stop=(t == T - 1))

            # ---- p-phase ------------------------------------------------
            def p_pre(c):
                dots(c, p2T, dpp[:, c * CH:(c + 1) * CH, :], 0, 2)
                nc.scalar.activation(e1c[c][:], dpp[:, c * CH:(c + 1) * CH, 0],
                                     EXP)

            def x_pre(c):
                dots(c, x2T, dpxc[c][:], 2, 4)
                nc.vector.tensor_tensor(out=skpc[c][:], in0=dpxc[c][:, :, 0],
                                        in1=mlog[:, c * CH:(c + 1) * CH],
                                        op=ADD)

            p_pre(0)
            p_pre(1)
            wsum(0, e1c[0], p2q, w1_ps, 257, 1)
            p_pre(2)
            wsum(1, e1c[1], p2q, w1_ps, 257, 1)
            p_pre(3)
            wsum(2, e1c[2], p2q, w1_ps, 257, 1)
            wsum(3, e1c[3], p2q, w1_ps, 257, 1)

            # ---- ctx1 chain ---------------------------------------------
            nc.vector.reciprocal(r1[:], w1_ps[0:1, 256:257])
            nc.vector.tensor_scalar_mul(out=r1n[:], in0=r1[:], scalar1=-1.0)
            nc.vector.tensor_copy(xb1rb[:], w1_ps[0:1, 0:256])
            xbT_ps = ps_sm.tile([P, 2], f32, tag="sm")
            for h in range(H):
                nc.tensor.matmul(xbT_ps[:, h:h + 1],
                                 xb1rb[0:1, h * P:(h + 1) * P],
                                 onesb[0:1, 0:1], start=True, stop=True)
            nc.scalar.copy(xb1T[:], xbT_ps[:])
            c1_ps = ps_sm.tile([1, 258], f32, tag="sm")
            for h in range(H):
                nc.tensor.matmul(c1_ps[:], xb1T[:, h:h + 1], rhs1s(h),
                                 start=(h == 0), stop=(h == 1))
            nc.vector.scalar_tensor_tensor(
                out=ctx1r[:], in0=c1_ps[0:1, 0:256], scalar=r1[:],
                in1=bv1r, op0=MUL, op1=ADD)
            nc.vector.scalar_tensor_tensor(
                out=gc1[0:1, 0:1], in0=c1_ps[0:1, 256:257], scalar=r1n[:],
                in1=cg1n, op0=MUL, op1=ADD)
            nc.vector.scalar_tensor_tensor(
                out=gc1[0:1, 1:2], in0=c1_ps[0:1, 257:258], scalar=r1[:],
                in1=cc21, op0=MUL, op1=ADD)
            gr_ps = ps_sm.tile([P, 2], f32, tag="sm")
            nc.tensor.matmul(gr_ps[:], onesf[:], gc1[:], start=True, stop=True)
            nc.scalar.copy(gcol[:, 0:2], gr_ps[:])
            nc.vector.tensor_copy(ctx1rb[:], ctx1r[:])
            bc_ps = ps_bc.tile([P, D], f32, tag="bc")
            nc.tensor.matmul(bc_ps[:], onesb[:], ctx1rb[:], start=True, stop=True)
            nc.scalar.copy(ctx1bc[:], bc_ps[:])

            # ---- x-phase ------------------------------------------------
            x_pre(0)
            x_pre(1)

            def x_post(c):
                q, r = divmod(c, 2)
                nc.scalar.activation(t1c[c][:], dpxc[c][:, :, 1], EXP,
                                     bias=gcol[:, 0:1], scale=-1.0)
                nc.vector.tensor_scalar_add(out=t1c[c][:], in0=t1c[c][:],
                                            scalar1=1.0)
                nc.vector.reciprocal(g1c[c][:], t1c[c][:])
                nc.vector.tensor_copy(
                    x2q[q][:, r * CH:(r + 1) * CH, 257], g1c[c][:])
                nc.vector.scalar_tensor_tensor(
                    out=skc[c][:], in0=g1c[c][:], scalar=gcol[:, 1:2],
                    in1=skpc[c][:], op0=MUL, op1=ADD)
                nc.scalar.activation(e2c[c][:], skc[c][:], EXP)
                for i in range(CH):
                    nc.vector.scalar_tensor_tensor(
                        out=xnc[c][:, i, :], in0=ctx1bc[:],
                        scalar=g1c[c][:, i:i + 1],
                        in1=x2q[q][:, r * CH + i, 0:256], op0=MUL, op1=ADD)
                nc.sync.dma_start(xo_d[:, c * CH * D:(c + 1) * CH * D],
                                  xnc[c][:])

            x_post(0)
            w2_ps = ps_w.tile([1, 258], f32, tag="w")
            x_pre(2)
            x_post(1)
            wsum(0, e2c[0], x2q, w2_ps, 258, 2)
            x_pre(3)
            x_post(2)
            wsum(1, e2c[1], x2q, w2_ps, 258, 2)
            x_post(3)
            wsum(2, e2c[2], x2q, w2_ps, 258, 2)
            wsum(3, e2c[3], x2q, w2_ps, 258, 2)

            # ---- ctx2 chain ---------------------------------------------
            nc.vector.reciprocal(r2[:], w2_ps[0:1, 256:257])
            nc.vector.tensor_scalar_mul(out=r2n[:], in0=r2[:], scalar1=-1.0)
            nc.scalar.copy(d22[:], w2_ps[0:1, 257:258])
            nc.vector.scalar_tensor_tensor(
                out=xb2r[:], in0=ctx1r[:], scalar=d22[:],
                in1=w2_ps[0:1, 0:256], op0=MUL, op1=ADD)
            xbT2_ps = ps_sm.tile([P, 2], f32, tag="sm")
            for h in range(H):
                nc.tensor.matmul(xbT2_ps[:, h:h + 1],
                                 xb2r[0:1, h * P:(h + 1) * P],
                                 onesf[0:1, 0:1], start=True, stop=True)
            nc.scalar.copy(xb2T[:], xbT2_ps[:])
            c2_ps = ps_sm.tile([1, 257], f32, tag="sm")
            for h in range(H):
                nc.tensor.matmul(c2_ps[:], xb2T[:, h:h + 1], rhs2s(h),
                                 start=(h == 0), stop=(h == 1))
            # gam2 first (g2 chain is long); ctx2 row next
            nc.vector.scalar_tensor_tensor(
                out=gc2[:], in0=c2_ps[0:1, 256:257], scalar=r2n[:],
                in1=cg2n, op0=MUL, op1=ADD)
            gr2_ps = ps_sm.tile([P, 1], f32, tag="sm")
            nc.tensor.matmul(gr2_ps[:], onesf[:], gc2[:], start=True, stop=True)
            nc.scalar.copy(gcol[:, 2:3], gr2_ps[:])
            nc.vector.scalar_tensor_tensor(
                out=ctx2r[:], in0=c2_ps[0:1, 0:256], scalar=r2[:],
                in1=bv2r, op0=MUL, op1=ADD)
            nc.vector.tensor_copy(ctx2rb[:], ctx2r[:])

            # ---- g2 + p_new ---------------------------------------------
            nc.scalar.activation(t2[:], dpp[:, :, 1], EXP,
                                 bias=gcol[:, 2:3], scale=-1.0)
            nc.vector.tensor_scalar_add(out=t2[:], in0=t2[:], scalar1=1.0)
            nc.vector.reciprocal(g2[:], t2[:])
            nc.vector.tensor_copy(g2b[:], g2[:])
            g2t_ps = ps_sm.tile([T, P], bfd, tag="sm")
            nc.tensor.transpose(g2t_ps[:], g2b[:], eye)
            nc.vector.tensor_copy(g2c[:], g2t_ps[:])
            nc.gpsimd.dma_start(g2t[:], g2c[:])

            bc2_ps = ps_bc.tile([P, D], f32, tag="bc")
            nc.tensor.matmul(bc2_ps[:], onesb[:], ctx2rb[:], start=True, stop=True)
            nc.scalar.copy(ctx2bc[:], bc2_ps[:])

            for t in range(T):
                q, r = divmod(t, Q)
                if t < PCMB_PE:
                    gp = ps_cmb.tile([P, D], f32, tag="cmb")
                    nc.tensor.matmul(gp[:], g2t[0:1, t * P:(t + 1) * P],
                                     ctx2rb[:], start=True, stop=False)
                    nc.tensor.matmul(gp[:], eye, p2q[q][:, r, 0:256],
                                     start=False, stop=True)
                    nc.scalar.copy(pnq[q][:, r, :], gp[:])
                else:
                    nc.vector.scalar_tensor_tensor(
                        out=pnq[q][:, r, :], in0=ctx2bc[:],
                        scalar=g2[:, t:t + 1],
                        in1=p2q[q][:, r, 0:256], op0=MUL, op1=ADD)
            nc.sync.dma_start(po_d[:, 0:Q * D], pnq[0][:])
            nc.sync.dma_start(po_d[:, Q * D:T * D], pnq[1][:])

    nc.finalize()

    # ---- host-side input prep ------------------------------------------
    u1, w1 = fold["ra1_u"], fold["ra1_w"]
    u2, w2 = fold["ra2_u"], fold["ra2_w"]
    Wv1, Wv2 = fold["ra1_Wv"], fold["ra2_Wv"]
    bv1, bv2 = fold["ra1_bv"], fold["ra2_bv"]
    wg11, wg21 = fold["ra1_wg1"], fold["ra2_wg1"]
    bg1, bg2 = fold["ra1_bg"], fold["ra2_bg"]

    Wv1h = Wv1 / 2.0
    R1 = np.concatenate([Wv1h, (Wv1h @ wg11)[:, None], (Wv1h @ u2)[:, None]],
                        axis=1)
    R2 = np.concatenate([Wv2, (Wv2 @ wg21)[:, None]], axis=1)
    ws = np.stack([u1 / 2.0, w2 / 2.0, u2, w1 / 2.0], axis=1)

    import ml_dtypes
    bf = ml_dtypes.bfloat16

    blobb = np.zeros((P, BLOBB_W), np.float64)
    blobb[:, WSTK0:WSTK0 + 8] = ws.reshape(H, P, 4).transpose(1, 0, 2).reshape(P, 8)
    blobb[:, RHS1_0:RHS1_0 + 516] = (
        R1.reshape(H, P, 258).transpose(1, 0, 2).reshape(P, 516))
    blobb[:, RHS2_0:RHS2_0 + 514] = (
        R2.reshape(H, P, 257).transpose(1, 0, 2).reshape(P, 514))
    blobb[:, EYE0:EYE0 + P] = np.eye(P)

    blobf0 = np.zeros((P, BLOBF_W), np.float32)
    blobf0[0, BV1_0:BV1_0 + D] = bv1
    blobf0[0, BV2_0:BV2_0 + D] = bv2
    blobf0[0, CG1N] = -(bv1 @ wg11 + bg1)
    blobf0[0, CC21] = bv1 @ u2
    blobf0[0, CG2N] = -(bv2 @ wg21 + bg2)

    shared = {"blobb": blobb.astype(bf)}

    x_np = np.asarray(inputs["x"], dtype=np.float64)
    p_np = np.asarray(inputs["p"], dtype=np.float64)
    m_np = np.asarray(inputs["mask"])
    ones_col = np.ones((P, T, 1), np.float64)
    zero_col = np.zeros((P, T, 1), np.float64)
    in_maps = []
    for b in range(NCORES):
        X2 = 2.0 * x_np[b]
        P2 = 2.0 * p_np[b]
        x2full = np.concatenate([_perm_rows(X2), ones_col, zero_col], axis=2)
        p2full = np.concatenate([_perm_rows(P2), ones_col], axis=2)
        bl = blobf0.copy()
        bl[:, MLOG0:MLOG0 + T] = np.where(
            m_np[b] == 0, NEG, 0.0).astype(np.float32).reshape(T, P).T
        im = dict(shared)
        im["blobf"] = bl
        im["x2"] = np.ascontiguousarray(x2full.reshape(P, T * 258)).astype(bf)
        im["p2"] = np.ascontiguousarray(p2full.reshape(P, T * 257)).astype(bf)
        im["x2T"] = np.ascontiguousarray(_perm_T(X2).reshape(P, H * T * P)).astype(bf)
        im["p2T"] = np.ascontiguousarray(_perm_T(P2).reshape(P, H * T * P)).astype(bf)
        in_maps.append(im)

    def post(results):
        def unperm(a):
            return np.ascontiguousarray(
                a.astype(np.float32).reshape(P, T, D).transpose(1, 0, 2)
                .reshape(N, D))
        x_new = np.stack([unperm(results[b]["x_out"]) for b in range(NCORES)])
        p_new = np.stack([unperm(results[b]["p_out"]) for b in range(NCORES)])
        return x_new, p_new

    return nc, in_maps, post


def kernel(**inputs):
    from concourse.bass_utils import run_bass_kernel_spmd

    nc, in_maps, post = build(inputs)
    res = run_bass_kernel_spmd(nc, in_maps, core_ids=list(range(NCORES)))
    return post(res.results)
